# revision 7
# baseline (speedup 1.0000x reference)
"""MoE feed-forward (top-2 routing, 8 experts) on 8 Trainium2 NeuronCores.

Expert-parallel sharding: host computes the (tiny) router + argsort
permutation exactly as the reference does, gathers each expert's token
chunk, and sends chunk e + expert e's weights to core e. Each core runs
a dense FFN: y = gelu(x @ w1) @ w2, scaled by the per-row gate weight.
Host then inverts the permutation and sums the top-2 contributions.

Device kernel (per core, SPMD):
  - x chunk [2048, 1024] f32 is PE-transposed on chip into xT tiles.
  - mm1 in float32r (tf32-class, 1 cycle/row): hT[f,c] += w1[d,f]^T.
  - gelu on ScalarE, output bf16.
  - mm2 in bf16 (w2 resident in SBUF): y[c,d] += hT[f,c]^T @ w2[f,d],
    accumulated over all 32 f-tiles in PSUM.
  - gate scale applied during the PSUM->SBUF drain on ScalarE.
"""

import numpy as np

# Static problem config
B, T, D, FF, E, TOP_K = 4, 2048, 1024, 4096, 8, 2
N = B * T                    # 8192 tokens
NE = N * TOP_K               # 16384 expanded rows
C_PER = NE // E              # 2048 rows per core / expert chunk
P = 128
FT = FF // P                 # 32 f-tiles
DT = D // P                  # 8 d-tiles
NCB = 4                      # c-blocks per core
CB = C_PER // NCB            # 512 tokens per c-block
NCT = C_PER // P             # 16 c-tiles of 128

_cache = {}
_EYE = np.eye(128, dtype=np.float32)


def _build_program(act_name="Gelu"):
    import concourse.bass as bass  # noqa: F401
    import concourse.mybir as mybir
    import concourse.tile as tile
    from concourse import bacc

    f32 = mybir.dt.float32
    f32r = mybir.dt.float32r
    bf16 = mybir.dt.bfloat16
    Act = mybir.ActivationFunctionType

    nc = bacc.Bacc("TRN2", num_devices=E)
    x_d = nc.dram_tensor("x", [C_PER, D], f32r, kind="ExternalInput")
    id_d = nc.dram_tensor("ident", [P, P], f32r, kind="ExternalInput")
    w1_d = nc.dram_tensor("w1r", [FT, P, DT, P], f32r, kind="ExternalInput")
    w2_d = nc.dram_tensor("w2b", [FF, D], bf16, kind="ExternalInput")
    sw_d = nc.dram_tensor("swt", [P, NCT], f32, kind="ExternalInput")
    y_d = nc.dram_tensor("y", [C_PER, D], f32, kind="ExternalOutput")

    with tile.TileContext(nc) as tc:
        with tc.tile_pool(name="const", bufs=1) as cpool, \
             tc.tile_pool(name="xin", bufs=2) as xin, \
             tc.tile_pool(name="xtp", bufs=1) as xtp, \
             tc.tile_pool(name="w1p", bufs=3) as w1p, \
             tc.tile_pool(name="w2p", bufs=1) as w2p, \
             tc.tile_pool(name="hp", bufs=1) as hp, \
             tc.tile_pool(name="yop", bufs=2) as yop, \
             tc.tile_pool(name="pst", bufs=2, space="PSUM") as pst, \
             tc.tile_pool(name="psh", bufs=2, space="PSUM") as psh, \
             tc.tile_pool(name="psy", bufs=4, space="PSUM") as psy:

            ident = cpool.tile([P, P], f32r, tag="ident")
            nc.sync.dma_start(out=ident, in_=id_d.ap())
            swt = cpool.tile([P, NCT], f32, tag="swt")
            nc.sync.dma_start(out=swt, in_=sw_d.ap())

            # Resident bf16 w2: 32 tiles [128, 1024]
            w2t = []
            for ft in range(FT):
                t = w2p.tile([P, D], bf16, tag=f"w2_{ft}", name=f"w2_{ft}")
                nc.sync.dma_start(out=t, in_=w2_d.ap()[ft * P:(ft + 1) * P, :])
                w2t.append(t)

            # Phase 0: load x and PE-transpose into xT [8][128, 2048]
            xt = [xtp.tile([P, C_PER], f32r, tag=f"xt_{d}", name=f"xt_{d}")
                  for d in range(DT)]
            for ct in range(NCT):
                xa = xin.tile([P, D], f32r, tag="xin", name="xa")
                nc.sync.dma_start(out=xa, in_=x_d.ap()[ct * P:(ct + 1) * P, :])
                for d in range(DT):
                    pt = pst.tile([P, P], f32r, tag="pst", name="pt")
                    nc.tensor.transpose(pt, xa[:, d * P:(d + 1) * P], ident)
                    nc.vector.tensor_copy(xt[d][:, ct * P:(ct + 1) * P], pt)

            # Main loop over c-blocks
            for cb in range(NCB):
                c0 = cb * CB
                ht = []
                for ft in range(FT):
                    w1t = w1p.tile([P, DT, P], f32r, tag="w1", name="w1t")
                    nc.sync.dma_start(out=w1t, in_=w1_d.ap()[ft])
                    hps = psh.tile([P, CB], f32, tag="psh", name="hps")
                    for d in range(DT):
                        nc.tensor.matmul(hps, w1t[:, d, :],
                                         xt[d][:, c0:c0 + CB],
                                         start=(d == 0), stop=(d == DT - 1))
                    h_t = hp.tile([P, CB], bf16, tag=f"h_{ft}", name=f"h_{ft}")
                    nc.scalar.activation(h_t, hps, getattr(Act, act_name))
                    ht.append(h_t)
                for ct in range(CB // P):
                    g = cb * (CB // P) + ct
                    yo = yop.tile([P, D], f32, tag="yo", name="yo")
                    for db in range(2):
                        d0 = db * (D // 2)
                        yps = psy.tile([P, D // 2], f32, tag="psy", name="yps")
                        for ft in range(FT):
                            nc.tensor.matmul(
                                yps, ht[ft][:, ct * P:(ct + 1) * P],
                                w2t[ft][:, d0:d0 + D // 2],
                                start=(ft == 0), stop=(ft == FT - 1))
                        nc.scalar.activation(yo[:, d0:d0 + D // 2], yps,
                                             Act.Copy, scale=swt[:, g:g + 1])
                    nc.sync.dma_start(out=y_d.ap()[g * P:(g + 1) * P, :],
                                      in_=yo)
    nc.compile()
    return nc


def _get_program():
    if "nc" not in _cache:
        _cache["nc"] = _build_program()
    return _cache["nc"]


def _routing(xf, router_w):
    """Replicate the reference gating bit-exactly where it matters.

    Returns (rev, sw): rev[i] = source token of sorted expanded row i,
    sw[i] = gate weight of sorted expanded row i.
    The top-k *selection* must match the reference exactly (it is
    discrete); we therefore compute the router logits with jax on the
    default platform when available, mirroring reference.py. The
    softmax / sort bookkeeping is continuous or exactly replicable in
    numpy.
    """
    topi = None
    topv = None
    try:
        import jax
        import jax.numpy as jnp
        logits = jnp.asarray(xf) @ jnp.asarray(router_w).T
        tv, ti = jax.lax.top_k(logits, TOP_K)
        topv = np.asarray(tv, dtype=np.float32)
        topi = np.asarray(ti)
    except Exception:
        logits = xf @ router_w.T
        # top-2 with jax tie-breaking (lower index wins)
        i0 = np.argmax(logits, axis=-1)
        v0 = np.take_along_axis(logits, i0[:, None], axis=-1)[:, 0]
        masked = logits.copy()
        np.put_along_axis(masked, i0[:, None], -np.inf, axis=-1)
        i1 = np.argmax(masked, axis=-1)
        v1 = np.take_along_axis(logits, i1[:, None], axis=-1)[:, 0]
        topi = np.stack([i0, i1], axis=-1)
        topv = np.stack([v0, v1], axis=-1).astype(np.float32)

    # softmax over the two gate logits, float32
    m = topv.max(axis=-1, keepdims=True)
    e = np.exp(topv - m, dtype=np.float32)
    topw = (e / e.sum(axis=-1, keepdims=True)).astype(np.float32)

    idx_flat = topi.reshape(-1)
    w_flat = topw.reshape(-1)
    # stable argsort of integer keys is uniquely determined by the keys
    sort_idx = np.argsort(idx_flat, kind="stable")
    src = np.repeat(np.arange(N), TOP_K)
    rev = src[sort_idx]
    sw = w_flat[sort_idx]
    return rev, sw, sort_idx


def kernel(x, router_w, w1, w2):
    from concourse import bass_utils

    xf = np.ascontiguousarray(x.reshape(-1, D), dtype=np.float32)
    rev, sw, sort_idx = _routing(xf, router_w)

    nc = _get_program()

    in_maps = []
    for e in range(E):
        rows = rev[e * C_PER:(e + 1) * C_PER]
        xc = np.ascontiguousarray(xf[rows])
        w1r = np.ascontiguousarray(
            w1[e].reshape(DT, P, FT, P).transpose(2, 1, 0, 3))
        import ml_dtypes
        w2b = np.ascontiguousarray(w2[e].astype(ml_dtypes.bfloat16))
        swt = np.ascontiguousarray(
            sw[e * C_PER:(e + 1) * C_PER].reshape(NCT, P).T)
        in_maps.append({"x": xc, "w1r": w1r, "w2b": w2b, "swt": swt,
                        "ident": _EYE})

    r = bass_utils.run_bass_kernel_spmd(nc, in_maps, core_ids=list(range(E)))
    _cache["last_result"] = r

    y_sorted = np.empty((NE, D), dtype=np.float32)
    for e in range(E):
        y_sorted[e * C_PER:(e + 1) * C_PER] = r.results[e]["y"]

    # invert the permutation and combine the top-2 contributions
    y_expanded = np.empty_like(y_sorted)
    y_expanded[sort_idx] = y_sorted
    out = y_expanded.reshape(N, TOP_K, D).sum(axis=1)
    return out.reshape(B, T, D)


# revision 8
# speedup vs baseline: 1.2242x; 1.2242x over previous
"""MoE feed-forward (top-2 routing, 8 experts) on 8 Trainium2 NeuronCores.

Expert-parallel sharding: host computes the (tiny) router + argsort
permutation exactly as the reference does, gathers each expert's token
chunk (pre-transposed to [D, C]), and sends chunk e + expert e's
weights to core e. Each core runs a dense FFN: y = gelu(x @ w1) @ w2,
scaled by the per-row gate weight. Host then inverts the permutation
and sums the top-2 contributions.

Device kernel (per core, SPMD):
  - mm1 in float32r (tf32-class, 1 cycle/row): hT[f,c] = w1[d,f]^T xT.
  - gelu on ScalarE, output bf16.
  - mm2 in bf16 (w2 resident in SBUF): y[c,d] += hT[f,c]^T @ w2[f,d],
    accumulated over all 32 f-tiles in PSUM.
  - gate scale applied during the PSUM->SBUF drain on ScalarE.
"""

import numpy as np

# Static problem config
B, T, D, FF, E, TOP_K = 4, 2048, 1024, 4096, 8, 2
N = B * T                    # 8192 tokens
NE = N * TOP_K               # 16384 expanded rows
C_PER = NE // E              # 2048 rows per core / expert chunk
P = 128
FT = FF // P                 # 32 f-tiles
DT = D // P                  # 8 d-tiles
NCB = 4                      # c-blocks per core
CB = C_PER // NCB            # 512 tokens per c-block
NCT = C_PER // P             # 16 c-tiles of 128

_cache = {}


def _build_program(act_name="Gelu"):
    import concourse.mybir as mybir
    import concourse.tile as tile
    from concourse import bacc

    f32 = mybir.dt.float32
    f32r = mybir.dt.float32r
    bf16 = mybir.dt.bfloat16
    Act = mybir.ActivationFunctionType

    nc = bacc.Bacc("TRN2", num_devices=E)
    xt_d = nc.dram_tensor("xt", [D, C_PER], f32r, kind="ExternalInput")
    w1_d = nc.dram_tensor("w1r", [FT, P, DT, P], f32r, kind="ExternalInput")
    w2_d = nc.dram_tensor("w2b", [FF, D], bf16, kind="ExternalInput")
    sw_d = nc.dram_tensor("swt", [P, NCT], f32, kind="ExternalInput")
    y_d = nc.dram_tensor("y", [C_PER, D], f32, kind="ExternalOutput")

    with tile.TileContext(nc) as tc:
        with tc.tile_pool(name="const", bufs=1) as cpool, \
             tc.tile_pool(name="xtp", bufs=1) as xtp, \
             tc.tile_pool(name="w1p", bufs=3) as w1p, \
             tc.tile_pool(name="w2p", bufs=1) as w2p, \
             tc.tile_pool(name="hp", bufs=1) as hp, \
             tc.tile_pool(name="yop", bufs=2) as yop, \
             tc.tile_pool(name="psh", bufs=3, space="PSUM") as psh, \
             tc.tile_pool(name="psy", bufs=4, space="PSUM") as psy:

            swt = cpool.tile([P, NCT], f32, tag="swt")
            nc.sync.dma_start(out=swt, in_=sw_d.ap())

            w2t = [None] * FT
            ht = [None] * FT
            xt = {}

            def load_xt(d, cb):
                t = xtp.tile([P, CB], f32r, tag=f"xt_{d}_{cb}",
                             name=f"xt_{d}_{cb}")
                nc.sync.dma_start(
                    out=t,
                    in_=xt_d.ap()[d * P:(d + 1) * P, cb * CB:(cb + 1) * CB])
                xt[(d, cb)] = t

            for cb in range(NCB):
                c0 = cb * CB
                for d in range(DT):
                    load_xt(d, cb)
                for ft in range(FT):
                    w1t = w1p.tile([P, DT, P], f32r, tag="w1", name="w1t")
                    nc.sync.dma_start(out=w1t, in_=w1_d.ap()[ft])
                    if cb == 0:
                        # stream resident w2 during the first mm1 phase
                        t = w2p.tile([P, D], bf16, tag=f"w2_{ft}",
                                     name=f"w2_{ft}")
                        nc.sync.dma_start(
                            out=t, in_=w2_d.ap()[ft * P:(ft + 1) * P, :])
                        w2t[ft] = t
                    hps = psh.tile([P, CB], f32, tag="psh", name="hps")
                    for d in range(DT):
                        nc.tensor.matmul(hps, w1t[:, d, :], xt[(d, cb)],
                                         start=(d == 0), stop=(d == DT - 1))
                    h_t = hp.tile([P, CB], bf16, tag=f"h_{ft}", name=f"h_{ft}")
                    nc.scalar.activation(h_t, hps, getattr(Act, act_name))
                    ht[ft] = h_t
                for ct in range(CB // P):
                    g = cb * (CB // P) + ct
                    yo = yop.tile([P, D], f32, tag="yo", name="yo")
                    for db in range(2):
                        d0 = db * (D // 2)
                        yps = psy.tile([P, D // 2], f32, tag="psy", name="yps")
                        for ft in range(FT):
                            nc.tensor.matmul(
                                yps, ht[ft][:, ct * P:(ct + 1) * P],
                                w2t[ft][:, d0:d0 + D // 2],
                                start=(ft == 0), stop=(ft == FT - 1))
                        nc.scalar.activation(yo[:, d0:d0 + D // 2], yps,
                                             Act.Copy, scale=swt[:, g:g + 1])
                    nc.sync.dma_start(out=y_d.ap()[g * P:(g + 1) * P, :],
                                      in_=yo)
    nc.compile()
    return nc


def _get_program():
    if "nc" not in _cache:
        _cache["nc"] = _build_program()
    return _cache["nc"]


def _routing(xf, router_w):
    """Replicate the reference gating bit-exactly where it matters.

    Returns (rev, sw, sort_idx). The top-k *selection* must match the
    reference exactly (it is discrete); we therefore compute the router
    logits with jax when available, mirroring reference.py. The softmax
    and sort bookkeeping is continuous or exactly replicable in numpy.
    """
    try:
        import jax
        import jax.numpy as jnp
        logits = jnp.asarray(xf) @ jnp.asarray(router_w).T
        tv, ti = jax.lax.top_k(logits, TOP_K)
        topv = np.asarray(tv, dtype=np.float32)
        topi = np.asarray(ti)
    except Exception:
        logits = xf @ router_w.T
        # top-2 with jax tie-breaking (lower index wins)
        i0 = np.argmax(logits, axis=-1)
        v0 = np.take_along_axis(logits, i0[:, None], axis=-1)[:, 0]
        masked = logits.copy()
        np.put_along_axis(masked, i0[:, None], -np.inf, axis=-1)
        i1 = np.argmax(masked, axis=-1)
        v1 = np.take_along_axis(logits, i1[:, None], axis=-1)[:, 0]
        topi = np.stack([i0, i1], axis=-1)
        topv = np.stack([v0, v1], axis=-1).astype(np.float32)

    # softmax over the two gate logits, float32
    m = topv.max(axis=-1, keepdims=True)
    e = np.exp(topv - m, dtype=np.float32)
    topw = (e / e.sum(axis=-1, keepdims=True)).astype(np.float32)

    idx_flat = topi.reshape(-1)
    w_flat = topw.reshape(-1)
    # stable argsort of integer keys is uniquely determined by the keys
    sort_idx = np.argsort(idx_flat, kind="stable")
    src = np.repeat(np.arange(N), TOP_K)
    rev = src[sort_idx]
    sw = w_flat[sort_idx]
    return rev, sw, sort_idx


def kernel(x, router_w, w1, w2):
    import ml_dtypes
    from concourse import bass_utils

    xf = np.ascontiguousarray(x.reshape(-1, D), dtype=np.float32)
    rev, sw, sort_idx = _routing(xf, router_w)

    nc = _get_program()

    in_maps = []
    for e in range(E):
        rows = rev[e * C_PER:(e + 1) * C_PER]
        xct = np.ascontiguousarray(xf[rows].T)
        w1r = np.ascontiguousarray(
            w1[e].reshape(DT, P, FT, P).transpose(2, 1, 0, 3))
        w2b = np.ascontiguousarray(w2[e].astype(ml_dtypes.bfloat16))
        swt = np.ascontiguousarray(
            sw[e * C_PER:(e + 1) * C_PER].reshape(NCT, P).T)
        in_maps.append({"xt": xct, "w1r": w1r, "w2b": w2b, "swt": swt})

    r = bass_utils.run_bass_kernel_spmd(nc, in_maps, core_ids=list(range(E)))
    _cache["last_result"] = r

    y_sorted = np.empty((NE, D), dtype=np.float32)
    for e in range(E):
        y_sorted[e * C_PER:(e + 1) * C_PER] = r.results[e]["y"]

    # invert the permutation and combine the top-2 contributions
    y_expanded = np.empty_like(y_sorted)
    y_expanded[sort_idx] = y_sorted
    out = y_expanded.reshape(N, TOP_K, D).sum(axis=1)
    return out.reshape(B, T, D)


# revision 11
# speedup vs baseline: 1.3369x; 1.0921x over previous
"""MoE feed-forward (top-2 routing, 8 experts) on 8 Trainium2 NeuronCores.

Expert-parallel sharding: host computes the (tiny) router + argsort
permutation exactly as the reference does, gathers each expert's token
chunk (pre-transposed to [D, C]), and sends chunk e + expert e's
weights to core e. Each core runs a dense FFN: y = gelu(x @ w1) @ w2,
scaled by the per-row gate weight. Host then inverts the permutation
and sums the top-2 contributions.

Device kernel (per core, SPMD):
  - mm1 in float32r (tf32-class, 1 cycle/row): hT[f,c] = w1[d,f]^T xT.
  - gelu on ScalarE, output bf16.
  - mm2 in bf16 (w2 resident in SBUF): y[c,d] += hT[f,c]^T @ w2[f,d],
    accumulated over all 32 f-tiles in PSUM.
  - gate scale applied during the PSUM->SBUF drain on ScalarE.
"""

import numpy as np

# Static problem config
B, T, D, FF, E, TOP_K = 4, 2048, 1024, 4096, 8, 2
N = B * T                    # 8192 tokens
NE = N * TOP_K               # 16384 expanded rows
C_PER = NE // E              # 2048 rows per core / expert chunk
P = 128
FT = FF // P                 # 32 f-tiles
DT = D // P                  # 8 d-tiles
NCB = 4                      # c-blocks per core
CB = C_PER // NCB            # 512 tokens per c-block
NCT = C_PER // P             # 16 c-tiles of 128

_cache = {}


def _build_program(act_name="Gelu"):
    import concourse.mybir as mybir
    import concourse.tile as tile
    from concourse import bacc

    f32 = mybir.dt.float32
    f32r = mybir.dt.float32r
    bf16 = mybir.dt.bfloat16
    Act = mybir.ActivationFunctionType

    nc = bacc.Bacc("TRN2", num_devices=E)
    xt_d = nc.dram_tensor("xt", [D, C_PER], f32r, kind="ExternalInput")
    w1_d = nc.dram_tensor("w1r", [FT, P, DT, P], f32r, kind="ExternalInput")
    w2_d = nc.dram_tensor("w2b", [FF, D], bf16, kind="ExternalInput")
    sw_d = nc.dram_tensor("swt", [P, NCT], f32, kind="ExternalInput")
    y_d = nc.dram_tensor("y", [C_PER, D], f32, kind="ExternalOutput")

    with tile.TileContext(nc) as tc:
        with tc.tile_pool(name="const", bufs=1) as cpool, \
             tc.tile_pool(name="xtp", bufs=1) as xtp, \
             tc.tile_pool(name="w1p", bufs=5) as w1p, \
             tc.tile_pool(name="w2p", bufs=1) as w2p, \
             tc.tile_pool(name="hp", bufs=1) as hp, \
             tc.tile_pool(name="yop", bufs=2) as yop, \
             tc.tile_pool(name="psh", bufs=3, space="PSUM") as psh, \
             tc.tile_pool(name="psy", bufs=4, space="PSUM") as psy:

            swt = cpool.tile([P, NCT], f32, tag="swt")
            nc.sync.dma_start(out=swt, in_=sw_d.ap())

            w2t = [None] * FT
            ht = [None] * FT
            xt = {}

            def load_xt(d, cb):
                t = xtp.tile([P, CB], f32r, tag=f"xt_{d}_{cb}",
                             name=f"xt_{d}_{cb}")
                nc.sync.dma_start(
                    out=t,
                    in_=xt_d.ap()[d * P:(d + 1) * P, cb * CB:(cb + 1) * CB])
                xt[(d, cb)] = t

            for d in range(DT):
                load_xt(d, 0)
            for cb in range(NCB):
                c0 = cb * CB
                for ft in range(FT):
                    # prefetch next c-block's xT mid-way through mm1
                    if ft == FT // 2 and cb + 1 < NCB:
                        for d in range(DT):
                            load_xt(d, cb + 1)
                    w1t = w1p.tile([P, DT, P], f32r, tag="w1", name="w1t")
                    nc.sync.dma_start(out=w1t, in_=w1_d.ap()[ft])
                    if cb == 0:
                        # stream resident w2 during the first mm1 phase
                        t = w2p.tile([P, D], bf16, tag=f"w2_{ft}",
                                     name=f"w2_{ft}")
                        nc.sync.dma_start(
                            out=t, in_=w2_d.ap()[ft * P:(ft + 1) * P, :])
                        w2t[ft] = t
                    hps = psh.tile([P, CB], f32, tag="psh", name="hps")
                    for d in range(DT):
                        nc.tensor.matmul(hps, w1t[:, d, :], xt[(d, cb)],
                                         start=(d == 0), stop=(d == DT - 1))
                    h_t = hp.tile([P, CB], bf16, tag=f"h_{ft}", name=f"h_{ft}")
                    nc.scalar.activation(h_t, hps, getattr(Act, act_name))
                    ht[ft] = h_t
                for ct in range(CB // P):
                    g = cb * (CB // P) + ct
                    yo = yop.tile([P, D], f32, tag="yo", name="yo")
                    for db in range(2):
                        d0 = db * (D // 2)
                        yps = psy.tile([P, D // 2], f32, tag="psy", name="yps")
                        for ft in range(FT):
                            nc.tensor.matmul(
                                yps, ht[ft][:, ct * P:(ct + 1) * P],
                                w2t[ft][:, d0:d0 + D // 2],
                                start=(ft == 0), stop=(ft == FT - 1))
                        nc.scalar.activation(yo[:, d0:d0 + D // 2], yps,
                                             Act.Copy, scale=swt[:, g:g + 1])
                    nc.sync.dma_start(out=y_d.ap()[g * P:(g + 1) * P, :],
                                      in_=yo)
    nc.compile()
    return nc


def _get_program():
    if "nc" not in _cache:
        _cache["nc"] = _build_program()
    return _cache["nc"]


def _routing(xf, router_w):
    """Replicate the reference gating bit-exactly where it matters.

    Returns (rev, sw, sort_idx). The top-k *selection* must match the
    reference exactly (it is discrete); we therefore compute the router
    logits with jax when available, mirroring reference.py. The softmax
    and sort bookkeeping is continuous or exactly replicable in numpy.
    """
    try:
        import jax
        import jax.numpy as jnp
        logits = jnp.asarray(xf) @ jnp.asarray(router_w).T
        tv, ti = jax.lax.top_k(logits, TOP_K)
        topv = np.asarray(tv, dtype=np.float32)
        topi = np.asarray(ti)
    except Exception:
        logits = xf @ router_w.T
        # top-2 with jax tie-breaking (lower index wins)
        i0 = np.argmax(logits, axis=-1)
        v0 = np.take_along_axis(logits, i0[:, None], axis=-1)[:, 0]
        masked = logits.copy()
        np.put_along_axis(masked, i0[:, None], -np.inf, axis=-1)
        i1 = np.argmax(masked, axis=-1)
        v1 = np.take_along_axis(logits, i1[:, None], axis=-1)[:, 0]
        topi = np.stack([i0, i1], axis=-1)
        topv = np.stack([v0, v1], axis=-1).astype(np.float32)

    # softmax over the two gate logits, float32
    m = topv.max(axis=-1, keepdims=True)
    e = np.exp(topv - m, dtype=np.float32)
    topw = (e / e.sum(axis=-1, keepdims=True)).astype(np.float32)

    idx_flat = topi.reshape(-1)
    w_flat = topw.reshape(-1)
    # stable argsort of integer keys is uniquely determined by the keys
    sort_idx = np.argsort(idx_flat, kind="stable")
    src = np.repeat(np.arange(N), TOP_K)
    rev = src[sort_idx]
    sw = w_flat[sort_idx]
    return rev, sw, sort_idx


def _ensure_axon_hooks():
    """Make `antenv.axon_hooks` importable so run_bass_kernel_spmd's
    trace path degrades gracefully (or works, if the axon boot shim is
    available) instead of crashing on ImportError."""
    try:
        import antenv.axon_hooks  # noqa: F401
        return
    except ImportError:
        pass
    import sys
    import types
    mod = types.ModuleType("antenv.axon_hooks")
    state = {"hook": None}
    mod.set_axon_ntff_profile_hook = lambda h: state.update(hook=h)
    mod.get_axon_ntff_profile_hook = lambda: state["hook"]
    try:
        import antenv
        sys.modules["antenv.axon_hooks"] = mod
        antenv.axon_hooks = mod
    except ImportError:
        return
    try:
        from trn_agent_boot.trn_boot import _ntff_profile_via_ctypes
        h = _ntff_profile_via_ctypes("/opt/axon/libaxon_pjrt.so")
        if h is not None:
            mod.set_axon_ntff_profile_hook(h)
            import concourse.bass_utils as bu
            bu.upload_artifacts = lambda tmpdir: "local://" + str(tmpdir)
    except Exception:
        pass


def kernel(x, router_w, w1, w2):
    import ml_dtypes
    from concourse import bass_utils
    _ensure_axon_hooks()

    xf = np.ascontiguousarray(x.reshape(-1, D), dtype=np.float32)
    rev, sw, sort_idx = _routing(xf, router_w)

    nc = _get_program()

    in_maps = []
    for e in range(E):
        rows = rev[e * C_PER:(e + 1) * C_PER]
        xct = np.ascontiguousarray(xf[rows].T)
        w1r = np.ascontiguousarray(
            w1[e].reshape(DT, P, FT, P).transpose(2, 1, 0, 3))
        w2b = np.ascontiguousarray(w2[e].astype(ml_dtypes.bfloat16))
        swt = np.ascontiguousarray(
            sw[e * C_PER:(e + 1) * C_PER].reshape(NCT, P).T)
        in_maps.append({"xt": xct, "w1r": w1r, "w2b": w2b, "swt": swt})

    r = bass_utils.run_bass_kernel_spmd(nc, in_maps, core_ids=list(range(E)))
    _cache["last_result"] = r

    y_sorted = np.empty((NE, D), dtype=np.float32)
    for e in range(E):
        y_sorted[e * C_PER:(e + 1) * C_PER] = r.results[e]["y"]

    # invert the permutation and combine the top-2 contributions
    y_expanded = np.empty_like(y_sorted)
    y_expanded[sort_idx] = y_sorted
    out = y_expanded.reshape(N, TOP_K, D).sum(axis=1)
    return out.reshape(B, T, D)
